# revision 13
# baseline (speedup 1.0000x reference)
"""GCN2 Trainium2 kernel: 3-layer GCN + FC head with BatchNorm, 8-core data-parallel.

Self-contained: hardcodes shapes from the problem spec.
  x [256, 128, 65] f32, adj_mat [256, 256] f32, W1 [63, 512], b1 [512],
  W2 [512, 512], b2 [512], W3 [512, 1024], b3 [1024], fcW1 [1024, 512],
  fcb1 [512], gamma [512], beta [512], fcW2 [512, 1], fcb2 [1] -> out [256, 1]

Sharding: batch 256 -> 32 samples per core on 8 cores; weights/adj replicated.
BatchNorm batch stats all-reduced across cores (one tiny [128,8] AllReduce).

v3 structure (DMA-lean + PE-dense):
  - No indirect DMA / DMA transposes / DRAM scratch: scatter via on-chip
    one-hot matmuls (iota + is_equal; ids unique per sample).
  - Zero-transpose layer chain:
      Xs (node-major) = OH^T F ; Y1T (feat-major) = Xs^T An ;
      X1 (node-major) = relu(Y1T^T W1p) ; ZT (feat-major) = X1^T An ;
      X2T (feat-major) = relu(W2^T ZT + b2) ; r = X2T . An[g,:] (DVE).
  - Samples processed in PAIRS so W2/ZT-consuming matmuls stream N=512
    (weight loads fully hidden; keeps the PE HAM-warm).
  - All PSUM tiles <= 1 bank, single deep pool for lookahead.
  - Evictions spread over Scalar/Vector/GpSimd; fcb1 dropped (BN cancels).
  - Feature-major head + per-partition BN math (no broadcasts, fast
    reciprocal on [128,4]); Lrelu fused via activation alpha.
"""
import os
import sys

if "/opt/trn_rl_repo" not in sys.path:
    sys.path.insert(0, "/opt/trn_rl_repo")

import numpy as np

import concourse.bass as bass
import concourse.mybir as mybir
import concourse.tile as tile
from concourse import bacc, bass_utils
from concourse.masks import make_identity

N_CORES = 8
BATCH, NODE, SEQ, FEAT = 256, 256, 128, 63   # FEAT = feature_num - 1
H1, H2, H3, FC = 512, 512, 1024, 512
BN_EPS = 1e-5
LEAKY = 0.01

F32 = mybir.dt.float32
BF16 = mybir.dt.bfloat16
I32 = mybir.dt.int32
AX = mybir.AxisListType
OP = mybir.AluOpType
ACTF = mybir.ActivationFunctionType


def build_nc(S: int):
    """Build the SPMD kernel for S samples per core."""
    nc = bacc.Bacc("TRN2", target_bir_lowering=False, debug=False,
                   num_devices=N_CORES)

    x_d = nc.dram_tensor("x", [S, SEQ, FEAT + 2], F32, kind="ExternalInput").ap()
    adj_d = nc.dram_tensor("adj_mat", [NODE, NODE], F32, kind="ExternalInput").ap()
    W1_d = nc.dram_tensor("W1", [FEAT, H1], F32, kind="ExternalInput").ap()
    b1_d = nc.dram_tensor("b1", [H1], F32, kind="ExternalInput").ap()
    W2_d = nc.dram_tensor("W2", [H1, H2], F32, kind="ExternalInput").ap()
    b2_d = nc.dram_tensor("b2", [H2], F32, kind="ExternalInput").ap()
    W3_d = nc.dram_tensor("W3", [H2, H3], F32, kind="ExternalInput").ap()
    b3_d = nc.dram_tensor("b3", [H3], F32, kind="ExternalInput").ap()
    fcW1_d = nc.dram_tensor("fcW1", [H3, FC], F32, kind="ExternalInput").ap()
    fcb1_d = nc.dram_tensor("fcb1", [FC], F32, kind="ExternalInput").ap()
    gamma_d = nc.dram_tensor("gamma", [FC], F32, kind="ExternalInput").ap()
    beta_d = nc.dram_tensor("beta", [FC], F32, kind="ExternalInput").ap()
    fcW2_d = nc.dram_tensor("fcW2", [FC, 1], F32, kind="ExternalInput").ap()
    fcb2_d = nc.dram_tensor("fcb2", [1], F32, kind="ExternalInput").ap()
    out_d = nc.dram_tensor("out", [S, 1], F32, kind="ExternalOutput").ap()

    with tile.TileContext(nc) as tc:
        _body(nc, tc, S, x_d, adj_d, W1_d, b1_d, W2_d, b2_d, W3_d, b3_d,
              fcW1_d, gamma_d, beta_d, fcW2_d, fcb2_d, out_d)
    nc.compile()
    return nc


def _body(nc, tc, S, x_d, adj_d, W1_d, b1_d, W2_d, b2_d, W3_d, b3_d,
          fcW1_d, gamma_d, beta_d, fcW2_d, fcb2_d, out_d):
    stage = int(os.environ.get("BISECT_STAGE", "0"))
    with tc.tile_pool(name="const", bufs=1) as cp, \
         tc.tile_pool(name="work", bufs=4) as wp, \
         tc.tile_pool(name="ps", bufs=6, space="PSUM") as ps, \
         tc.tile_pool(name="psb", bufs=2, space="PSUM") as psb, \
         tc.tile_pool(name="dram", bufs=1, space="DRAM") as dp:

        # ---------------- input DMAs (plain f32, contiguous runs) -----------
        Fall = cp.tile([128, S, FEAT + 2], F32)  # [seq, sample, feat]
        nc.scalar.dma_start(Fall[:], x_d.rearrange("b j f -> j b f"))
        A0 = cp.tile([128, 2, NODE], F32)        # chunk c = rows 128c..128c+127
        nc.sync.dma_start(A0[:], adj_d.rearrange("(c p) n -> p c n", p=128))
        W1f = cp.tile([FEAT, H1], F32)
        nc.sync.dma_start(W1f[:], W1_d[:])
        b2raw = cp.tile([4, 128], F32)
        nc.sync.dma_start(b2raw[:], b2_d.rearrange("(c p) -> c p", p=128))
        b3raw = cp.tile([8, 128], F32)
        nc.sync.dma_start(b3raw[:], b3_d.rearrange("(c p) -> c p", p=128))
        gbraw = cp.tile([4, 3, 128], F32)        # gamma | beta | fcW2
        nc.sync.dma_start(gbraw[:, 0, :], gamma_d.rearrange("(c p) -> c p", p=128))
        nc.sync.dma_start(gbraw[:, 1, :], beta_d.rearrange("(c p) -> c p", p=128))
        nc.sync.dma_start(gbraw[:, 2, :], fcW2_d.rearrange("(c p) 1 -> c p", p=128))
        fcb2r = cp.tile([1, 1], F32)
        nc.sync.dma_start(fcb2r[:], fcb2_d[None, :])
        # bulk weights on the scalar HWDGE ring (overlap with loop)
        W2f = cp.tile([128, 4, H2], F32)
        nc.sync.dma_start(W2f[:], W2_d.rearrange("(c p) h -> p c h", p=128))
        W3f = cp.tile([128, 4, H3], F32)
        nc.scalar.dma_start(W3f[:], W3_d.rearrange("(c p) h -> p c h", p=128))
        fcW1f = cp.tile([128, 8, FC], F32)
        nc.scalar.dma_start(fcW1f[:], fcW1_d.rearrange("(c p) h -> p c h", p=128))

        # ---------------- on-chip weight casts f32 -> bf16 ------------------
        W1p = cp.tile([96, H1], BF16)
        nc.vector.memset(W1p[:], 0.0)
        nc.vector.tensor_copy(W1p[0:FEAT, :], W1f[:])
        nc.gpsimd.dma_start(W1p[64:65, :], b1_d[None, :])   # tiny cast-DMA
        W2sb = cp.tile([128, 4, H2], BF16)
        nc.vector.tensor_copy(W2sb[:], W2f[:])
        W3sb = cp.tile([128, 4, H3], BF16)
        nc.scalar.activation(W3sb[:], W3f[:], ACTF.Copy)
        fcW1sb = cp.tile([128, 8, FC], BF16)
        nc.scalar.activation(fcW1sb[:], fcW1f[:], ACTF.Copy)

        ident = cp.tile([128, 128], F32)
        make_identity(nc, ident[:])

        # feature-on-partition constants via PE transpose
        def _tr(raw, n, dtype):
            p = ps.tile([128, n], F32, tag="w")
            nc.tensor.transpose(p[:], raw, ident[0:n, 0:n])
            t = cp.tile([128, n], dtype)
            nc.vector.tensor_copy(t[:], p[:])
            return t

        b2c = _tr(b2raw[:], 4, F32)
        b3c = _tr(b3raw[:], 8, F32)
        gamc = _tr(gbraw[:, 0, :], 4, F32)
        betc = _tr(gbraw[:, 1, :], 4, F32)
        fcW2c = _tr(gbraw[:, 2, :], 4, BF16)

        # ---------------- iotas / one-hot helpers ---------------------------
        iotaRowI = cp.tile([128, NODE], I32)
        nc.gpsimd.iota(iotaRowI[:], pattern=[[1, NODE]], base=0,
                       channel_multiplier=0)
        iotaRowF = cp.tile([128, NODE], F32)
        nc.vector.tensor_copy(iotaRowF[:], iotaRowI[:])
        iotaCNI = cp.tile([128, 2], I32)
        nc.gpsimd.iota(iotaCNI[:], pattern=[[128, 2]], base=0,
                       channel_multiplier=1)
        iotaCNF = cp.tile([128, 2], F32)
        nc.vector.tensor_copy(iotaCNF[:], iotaCNI[:])
        e127 = cp.tile([128, 1], F32)   # one-hot of partition 127
        nc.vector.tensor_scalar(out=e127[:], in0=iotaCNF[:, 0:1],
                                scalar1=float(SEQ - 1), scalar2=None,
                                op0=OP.is_equal)
        Id32b = cp.tile([S, S], BF16)
        make_identity(nc, Id32b[:])
        epsc = cp.tile([128, 1], F32)
        nc.gpsimd.memset(epsc[:], BN_EPS)

        # ---------------- adjacency normalization ---------------------------
        # An = diag(dis) (A + I) diag(dis),  dis = 1/sqrt(rowsum(A) + 1)
        dis = cp.tile([128, 2], F32)
        rs = cp.tile([128, 2], F32)
        for c in range(2):
            nc.vector.tensor_reduce(rs[:, c:c + 1], A0[:, c, :], axis=AX.X, op=OP.add)
        sq = cp.tile([128, 2], F32)
        nc.scalar.activation(sq[:], rs[:], ACTF.Sqrt, bias=1.0)
        nc.vector.reciprocal(dis[:], sq[:])
        dis2 = cp.tile([128, 2], F32)
        nc.vector.tensor_tensor(dis2[:], dis[:], dis[:], op=OP.mult)
        Csc = cp.tile([128, 2, NODE], F32)
        for c in range(2):
            nc.vector.tensor_scalar_mul(Csc[:, c, :], A0[:, c, :],
                                        dis[:, c:c + 1])
        Anb = cp.tile([128, 2, NODE], BF16)      # normalized adjacency, bf16
        for cd in range(2):      # destination row chunk
            for cs in range(2):  # source row chunk
                pT = ps.tile([128, 128], F32, tag="w")
                nc.tensor.transpose(pT[:], Csc[:, cs, 128 * cd:128 * (cd + 1)],
                                    ident[:])
                nc.scalar.activation(Anb[:, cd, 128 * cs:128 * (cs + 1)], pT[:],
                                     ACTF.Copy, scale=dis[:, cd:cd + 1])
        diagb = cp.tile([128, 2, NODE], BF16)
        for c in range(2):
            nc.gpsimd.affine_select(
                out=diagb[:, c, :], in_=dis2[:, c:c + 1].to_broadcast([128, NODE]),
                pattern=[[-1, NODE]], compare_op=OP.is_equal, fill=0.0,
                base=128 * c, channel_multiplier=1)
            nc.vector.tensor_tensor(Anb[:, c, :], Anb[:, c, :], diagb[:, c, :],
                                    op=OP.add)

        if stage == 1:
            nc.sync.dma_start(out_d.rearrange("b 1 -> 1 b"), Anb[0:1, 0, 0:S])
            return

        # ---------------- station ids, g-row one-hots ------------------------
        sidF = cp.tile([128, S], F32)
        nc.vector.tensor_copy(sidF[:], Fall[:, :, FEAT:FEAT + 1].rearrange("p b 1 -> p b"))
        pgs = ps.tile([128, S], F32, tag="w")
        nc.tensor.matmul(pgs[:], lhsT=e127[:, 0:1].to_broadcast([128, 128]),
                         rhs=sidF[:], start=True, stop=True)
        gsidB = cp.tile([128, S], F32)
        nc.vector.tensor_copy(gsidB[:], pgs[:])
        gOH = cp.tile([128, 2, S], BF16)         # node-major one-hot of g_sid
        for c in range(2):
            nc.vector.tensor_scalar(out=gOH[:, c, :], in0=gsidB[:],
                                    scalar1=iotaCNF[:, c:c + 1], scalar2=None,
                                    op0=OP.is_equal)
        pAR = ps.tile([S, NODE], F32, tag="w")
        for c in range(2):
            nc.tensor.matmul(pAR[:], lhsT=gOH[:, c, :], rhs=Anb[:, c, :],
                             start=(c == 0), stop=(c == 1))
        AnRowsB = cp.tile([S, NODE], BF16)
        nc.vector.tensor_copy(AnRowsB[:], pAR[:])

        if stage == 2:
            nc.sync.dma_start(out_d.rearrange("b 1 -> 1 b"), AnRowsB[0:1, 0:S])
            return

        R = cp.tile([128, 4, S], F32)    # r vectors, feature-major
        if stage:
            nc.vector.memset(R[:], 0.0)

        # precompute all sample one-hots + bf16 features (lead-in, V idle)
        OHall = cp.tile([128, S, NODE], BF16)
        for b in range(S):
            nc.vector.tensor_scalar(out=OHall[:, b, :], in0=iotaRowF[:],
                                    scalar1=sidF[:, b:b + 1], scalar2=None,
                                    op0=OP.is_equal)
        Fball = cp.tile([128, S, FEAT], BF16)
        nc.gpsimd.tensor_copy(Fball[:, 0:S // 2, :], Fall[:, 0:S // 2, 0:FEAT])
        nc.gpsimd.tensor_copy(Fball[:, S // 2:S, :], Fall[:, S // 2:S, 0:FEAT])

        # ---------------- per-PAIR pipeline (r software-pipelined) ----------
        pend = None     # (b0_prev, X2bp_prev, [pB0, pB1]) from previous pair

        def emit_r(junk, picks):
            b0p, X2p, pBp = pend
            for (m, s) in picks:
                nc.vector.scalar_tensor_tensor(
                    out=junk[:], in0=X2p[:, m, s, :], scalar=1.0,
                    in1=pBp[s][:], op0=OP.mult, op1=OP.mult,
                    accum_out=R[:, m, b0p + s:b0p + s + 1])

        for i in range(S // 2):
            b0 = 2 * i
            junkV = wp.tile([128, NODE], BF16, tag="junkV")

            # scatter: Xs[m, f] = sum_j OH[j, m] F[j, f]  (node-major)
            Xsbp = wp.tile([128, 2, 2, FEAT], BF16, tag="Xsb")   # [p, s, c, f]
            for s in range(2):
                pXs = ps.tile([128, 2, FEAT], F32, tag="w")
                for c in range(2):
                    nc.tensor.matmul(pXs[:, c, :],
                                     lhsT=OHall[:, b0 + s, 128 * c:128 * (c + 1)],
                                     rhs=Fball[:, b0 + s, :], start=True, stop=True)
                if s == 0:
                    nc.vector.tensor_copy(Xsbp[:, s, :, :], pXs[:])
                else:
                    nc.scalar.activation(Xsbp[:, s, :, :], pXs[:], ACTF.Copy)

            # L1 graph-mult: Y1T[f, n] = sum_m Xs[m, f] An[m, n] (feat-major)
            pY1 = ps.tile([96, 2, NODE], F32, tag="w")
            for s in range(2):
                for c in range(2):
                    nc.tensor.matmul(pY1[0:FEAT, s, :], lhsT=Xsbp[:, s, c, :],
                                     rhs=Anb[:, c, :],
                                     start=(c == 0), stop=(c == 1))
            Y1Tp = wp.tile([96, 2, NODE], BF16, tag="Y1T")
            nc.gpsimd.memset(Y1Tp[32:64, :, :], 0.0)
            nc.gpsimd.memset(Y1Tp[64:96, :, :], 0.0)
            nc.scalar.activation(Y1Tp[0:FEAT, :, :], pY1[0:FEAT, :, :], ACTF.Copy)
            nc.gpsimd.memset(Y1Tp[64:65, :, :], 1.0)
            if pend is not None:
                emit_r(junkV, [(0, 0), (1, 0)])

            if stage == 31:
                if i == S // 2 - 1:
                    nc.sync.dma_start(out_d.rearrange("b 1 -> 1 b"), Y1Tp[0:1, 0, 0:S])
                continue

            # W1 (+b1 via ones row), relu -> X1 node-major [128, t, s, 512]
            X1bp = wp.tile([128, 2, 2, H1], BF16, tag="X1")
            for s in range(2):
                for t in range(2):
                    pX1 = ps.tile([128, H1], F32, tag="w")
                    nc.tensor.matmul(pX1[:], lhsT=Y1Tp[:, s, 128 * t:128 * (t + 1)],
                                     rhs=W1p[:], start=True, stop=True)
                    if s == 0 and t == 0:
                        nc.vector.tensor_scalar_max(X1bp[:, t, s, :], pX1[:], 0.0)
                    else:
                        nc.scalar.activation(X1bp[:, t, s, :], pX1[:], ACTF.Relu)

            if pend is not None:
                emit_r(junkV, [(2, 0), (3, 0)])

            if stage == 32:
                if i == S // 2 - 1:
                    nc.sync.dma_start(out_d.rearrange("b 1 -> 1 b"), X1bp[0:1, 0, 0, 0:S])
                continue

            # L2 graph-mult, feature-major: ZT[h, n] = sum_m X1[m, h] An[m, n]
            ZTbp = wp.tile([128, 4, 2, NODE], BF16, tag="ZT")   # [p, mb, s, n]
            for mb in range(4):
                pZT = ps.tile([128, 2, NODE], F32, tag="w")
                for s in range(2):
                    for k in range(2):
                        nc.tensor.matmul(pZT[:, s, :],
                                         lhsT=X1bp[:, k, s, 128 * mb:128 * (mb + 1)],
                                         rhs=Anb[:, k, :],
                                         start=(k == 0), stop=(k == 1))
                if mb % 2 == 0:
                    nc.scalar.activation(ZTbp[:, mb, :, :], pZT[:], ACTF.Copy)
                else:
                    nc.vector.tensor_copy(ZTbp[:, mb, :, :], pZT[:])

            if pend is not None:
                emit_r(junkV, [(0, 1), (1, 1)])

            if stage == 33:
                if i == S // 2 - 1:
                    nc.sync.dma_start(out_d.rearrange("b 1 -> 1 b"), ZTbp[0:1, 0, 0, 0:S])
                continue

            # W2 + b2, relu -> X2T feature-major [p, mb, s, n], N=512 matmuls
            X2bp = wp.tile([128, 4, 2, NODE], BF16, tag="X2")
            for mb in range(4):
                pX2 = ps.tile([128, 2, NODE], F32, tag="w")
                for c in range(4):
                    nc.tensor.matmul(pX2[:],
                                     lhsT=W2sb[:, c, 128 * mb:128 * (mb + 1)],
                                     rhs=ZTbp[:, c, :, :],
                                     start=(c == 0), stop=(c == 3))
                if mb % 2 == 0:
                    nc.scalar.activation(X2bp[:, mb, :, :], pX2[:], ACTF.Relu,
                                         bias=b2c[:, mb:mb + 1])
                else:
                    nc.vector.tensor_scalar(
                        out=X2bp[:, mb, :, :], in0=pX2[:],
                        scalar1=b2c[:, mb:mb + 1], scalar2=0.0,
                        op0=OP.add, op1=OP.max)

            if pend is not None:
                emit_r(junkV, [(2, 1), (3, 1)])

            if stage == 34:
                if i == S // 2 - 1:
                    nc.sync.dma_start(out_d.rearrange("b 1 -> 1 b"), X2bp[0:1, 0, 0, 0:S])
                continue

            # broadcast An[g_b, :] rows for this pair; r-ops run next iter
            pBs = []
            for s in range(2):
                b = b0 + s
                pB = psb.tile([128, NODE], F32, tag="b")
                nc.tensor.matmul(pB[:], lhsT=Id32b[:, b:b + 1].to_broadcast([S, 128]),
                                 rhs=AnRowsB[:], start=True, stop=True)
                pBs.append(pB)
            pend = (b0, X2bp, pBs)

        if pend is not None:
            junkF = wp.tile([128, NODE], BF16, tag="junkV")
            emit_r(junkF, [(m, s) for s in range(2) for m in range(4)])

        if stage == 3 or (30 < stage < 40):
            nc.sync.dma_start(out_d.rearrange("b 1 -> 1 b"), R[0:1, 0, 0:S])
            return

        # ---------------- batched head (feature-major) -----------------------
        Rbb = cp.tile([128, 4, S], BF16)
        nc.vector.tensor_copy(Rbb[:], R[:])
        # G3 = relu(W3^T r + b3), feature-major [128, 8, S]
        G3 = cp.tile([128, 8, S], BF16)
        for mb in range(8):
            pG = ps.tile([128, S], F32, tag="w")
            for c in range(4):
                nc.tensor.matmul(pG[:], lhsT=W3sb[:, c, 128 * mb:128 * (mb + 1)],
                                 rhs=Rbb[:, c, :], start=(c == 0), stop=(c == 3))
            if mb % 2 == 0:
                nc.scalar.activation(G3[:, mb, :], pG[:], ACTF.Relu,
                                     bias=b3c[:, mb:mb + 1])
            else:
                nc.vector.tensor_scalar(
                    out=G3[:, mb, :], in0=pG[:],
                    scalar1=b3c[:, mb:mb + 1], scalar2=0.0,
                    op0=OP.add, op1=OP.max)

        # H = fcW1^T G3, feature-major [128, 4, S] f32 (fcb1 cancelled by BN)
        Hf = cp.tile([128, 4, S], F32)
        for mb in range(4):
            pH = ps.tile([128, S], F32, tag="w")
            for c in range(8):
                nc.tensor.matmul(pH[:], lhsT=fcW1sb[:, c, 128 * mb:128 * (mb + 1)],
                                 rhs=G3[:, c, :], start=(c == 0), stop=(c == 7))
            if mb % 2 == 0:
                nc.scalar.activation(Hf[:, mb, :], pH[:], ACTF.Identity)
            else:
                nc.vector.tensor_copy(Hf[:, mb, :], pH[:])

        # local BN stats: cols 0-3 sums, 4-7 sum-squares
        stats = cp.tile([128, 8], F32)
        sjunk = cp.tile([128, S], F32)
        for m in range(4):
            nc.vector.tensor_reduce(stats[:, m:m + 1], Hf[:, m, :], axis=AX.X,
                                    op=OP.add)
            nc.scalar.activation(sjunk[:], Hf[:, m, :], ACTF.Square,
                                 accum_out=stats[:, 4 + m:5 + m])

        if stage == 4:
            nc.sync.dma_start(out_d.rearrange("b 1 -> 1 b"), stats[0:1, 0:S])
            return

        cc_in = dp.tile([128, 8], F32)
        cc_out = dp.tile([128, 8], F32)
        nc.sync.dma_start(cc_in[:], stats[:])
        nc.gpsimd.collective_compute(
            "AllReduce", OP.add, replica_groups=[list(range(N_CORES))],
            ins=[cc_in.opt()], outs=[cc_out.opt()])
        statsG = cp.tile([128, 8], F32)
        nc.sync.dma_start(statsG[:], cc_out[:])

        if stage == 5:
            nc.sync.dma_start(out_d.rearrange("b 1 -> 1 b"), statsG[0:1, 0:S])
            return

        # BN math, per-partition [128, 4]
        inv_n = 1.0 / (S * N_CORES)
        mean = cp.tile([128, 4], F32)
        nc.vector.tensor_scalar_mul(mean[:], statsG[:, 0:4], inv_n)
        ex2 = cp.tile([128, 4], F32)
        nc.vector.tensor_scalar_mul(ex2[:], statsG[:, 4:8], inv_n)
        var = cp.tile([128, 4], F32)
        nc.vector.tensor_tensor(var[:], mean[:], mean[:], op=OP.mult)
        nc.vector.tensor_tensor(var[:], ex2[:], var[:], op=OP.subtract)
        sd = cp.tile([128, 4], F32)
        nc.scalar.activation(sd[:], var[:], ACTF.Sqrt, bias=epsc[:, 0:1])
        rstd = cp.tile([128, 4], F32)
        nc.vector.reciprocal(rstd[:], sd[:])
        scl = cp.tile([128, 4], F32)
        nc.vector.tensor_tensor(scl[:], gamc[:], rstd[:], op=OP.mult)
        sft = cp.tile([128, 4], F32)
        nc.vector.tensor_tensor(sft[:], mean[:], scl[:], op=OP.mult)
        nc.vector.tensor_tensor(sft[:], betc[:], sft[:], op=OP.subtract)

        # Hl = leaky(H*scl + sft); out = sigmoid(fcW2^T Hl + fcb2)
        Hn = cp.tile([128, 4, S], F32)
        for m in range(4):
            nc.scalar.activation(Hn[:, m, :], Hf[:, m, :], ACTF.Identity,
                                 scale=scl[:, m:m + 1], bias=sft[:, m:m + 1])
        Hl = cp.tile([128, 4, S], BF16)
        nc.vector.scalar_tensor_tensor(
            out=Hl[:], in0=Hn[:], scalar=LEAKY, in1=Hn[:],
            op0=OP.mult, op1=OP.max)
        pO = ps.tile([1, S], F32, tag="w")
        for c in range(4):
            nc.tensor.matmul(pO[:], lhsT=fcW2c[:, c:c + 1], rhs=Hl[:, c, :],
                             start=(c == 0), stop=(c == 3))
        osig = cp.tile([1, S], F32)
        nc.scalar.activation(osig[:], pO[:], ACTF.Sigmoid, bias=fcb2r[:, 0:1])
        nc.sync.dma_start(out_d.rearrange("b 1 -> 1 b"), osig[:])


_NC_CACHE = {}
_LAST_RESULT = None


def _get_nc(S: int):
    if S not in _NC_CACHE:
        _NC_CACHE[S] = build_nc(S)
    return _NC_CACHE[S]


def kernel(**inputs) -> np.ndarray:
    S = BATCH // N_CORES
    nc = _get_nc(S)
    full_x = np.ascontiguousarray(inputs["x"], dtype=np.float32)
    shared = {}
    for k in ("adj_mat", "W1", "b1", "W2", "b2", "W3", "b3", "fcW1", "fcb1",
              "gamma", "beta", "fcW2", "fcb2"):
        shared[k] = np.ascontiguousarray(inputs[k], dtype=np.float32)
    in_maps = []
    for c in range(N_CORES):
        m = dict(shared)
        m["x"] = np.ascontiguousarray(full_x[c * S:(c + 1) * S])
        in_maps.append(m)
    res = bass_utils.run_bass_kernel_spmd(
        nc, in_maps, core_ids=list(range(N_CORES)))
    global _LAST_RESULT
    _LAST_RESULT = res
    out = np.concatenate([res.results[c]["out"] for c in range(N_CORES)], axis=0)
    return out.astype(np.float32)


if __name__ == "__main__":
    print("building...")
    nc = _get_nc(BATCH // N_CORES)
    print("built ok")
